# revision 4
# baseline (speedup 1.0000x reference)
"""Trainium2 Bass kernel for nn_Attention (B=2, S=2048, D=512, H=8).

Sharding: 8 cores = 2 batches x 4 head-groups (2 heads each).

Algebraic fusions (exact, host-side weight preprocessing in f64):
  W_full = W_multi @ W_sep  (the two projection layers collapse into one),
  G_h    = Wq_h^T @ Wk_h    so  S_h = (x G_h) x^T   (k-projection vanishes),
  Wvp_h  = (Wres_h @ Wv_h)^T so PV directly yields the output partial
           (restore matmul vanishes).
Bias terms: q-side/constant bias cancels inside softmax; the surviving
k-side term (x Wk^T bq)/sqrt(D) is a per-sk vector folded into the exp's
per-partition bias; V'-bias is a broadcast add; b_res is added on host.

Device compute per core (bf16 matmuls, f32 PSUM, feature-on-partition
layouts, zero on-device transposes):
  qtT = G^T xT                      [D, S]   (per head)
  V'  = x Wvp (+bias)               [S, D]   natural layout
  t3  = x g                         [S]      (exp bias column)
  ST  = x qt^T  -> E = exp(ST/sqrt(D) + t3)  [sk, sq] chunks
  den = E^T ones ; out_h = (E^T V') * 1/den  accumulated over heads into
  out [S, D] f32 (natural layout), summed over cores on host + b_res.
"""

import numpy as np

P = 128
B = 2
S = 2048
D = 512          # word dim == head dim
H = 8            # total heads
E3 = 3 * D       # 1536
NHL = 2          # local heads per core
NC = 8           # cores
CH = 512         # sq chunk width
NCH = S // CH    # 4
NT = S // P      # 16 sk tiles
KD = D // P      # 4
INV_SQRT_D = 1.0 / float(np.sqrt(np.float32(D)))

_CACHE = {}


def _build_nc():
    import concourse.mybir as mybir
    import concourse.tile as tile
    from concourse import bacc

    dt = mybir.dt
    BF = dt.bfloat16
    F32 = dt.float32
    Act = mybir.ActivationFunctionType
    Alu = mybir.AluOpType

    nc = bacc.Bacc("TRN2", target_bir_lowering=False, debug=False, num_devices=NC)

    xT_d = nc.declare_dram_parameter("xT", [D, S], BF, isOutput=False)
    g_d = nc.declare_dram_parameter("G", [NHL, D, D], BF, isOutput=False)
    wvp_d = nc.declare_dram_parameter("WvpT", [NHL, D, D], BF, isOutput=False)
    gv_d = nc.declare_dram_parameter("gvec", [NHL, D], BF, isOutput=False)
    bfv_d = nc.declare_dram_parameter("bfv", [NHL, D], F32, isOutput=False)
    out_d = nc.declare_dram_parameter("out", [S, D], F32, isOutput=True)

    with tile.TileContext(nc) as tc:
        with (
            tc.tile_pool(name="w", bufs=1) as wp,
            tc.tile_pool(name="psum", bufs=1, space="PSUM") as pp,
        ):
            ones_sb = wp.tile([P, 1], BF, tag="ones")
            nc.vector.memset(ones_sb[:], 1.0)

            # xT: d_in on partitions; DMA'd in column chunks for fast start
            xT = []
            for kd in range(KD):
                xT.append(wp.tile([P, S], BF, tag=f"xT{kd}", name=f"xT{kd}"))
            for s in range(NCH):
                for kd in range(KD):
                    nc.sync.dma_start(
                        xT[kd][:, s * CH : (s + 1) * CH],
                        xT_d[kd * P : (kd + 1) * P, s * CH : (s + 1) * CH],
                    )

            # per-head weights
            Gt, Wvp, gcol, bfvb = [], [], [], []
            for h in range(NHL):
                gt = []
                for kd in range(KD):
                    t = wp.tile([P, D], BF, tag=f"G{h}_{kd}", name=f"G{h}_{kd}")
                    nc.sync.dma_start(t[:], g_d[h, kd * P : (kd + 1) * P, :])
                    gt.append(t)
                Gt.append(gt)
                wv = []
                for kd in range(KD):
                    t = wp.tile([P, D], BF, tag=f"Wvp{h}_{kd}", name=f"Wvp{h}_{kd}")
                    nc.sync.dma_start(t[:], wvp_d[h, kd * P : (kd + 1) * P, :])
                    wv.append(t)
                Wvp.append(wv)
                gc = wp.tile([P, KD], BF, tag=f"gcol{h}", name=f"gcol{h}")
                nc.sync.dma_start(gc[:], gv_d[h, :].rearrange("(k p) -> p k", p=P))
                gcol.append(gc)
                brow = wp.tile([1, D], F32, tag=f"bfvrow{h}", name=f"bfvrow{h}")
                nc.sync.dma_start(brow[:], bfv_d[h, :].unsqueeze(0))
                bb = wp.tile([P, D], F32, tag=f"bfvb{h}", name=f"bfvb{h}")
                nc.gpsimd.partition_broadcast(bb[:], brow[:])
                bfvb.append(bb)

            # out_acc: head-0 partial, f32 (16 tiles of [128, D])
            out_acc = []
            for st in range(NT):
                out_acc.append(
                    wp.tile([P, D], F32, tag=f"oacc{st}", name=f"oacc{st}")
                )

            for h in range(NHL):
                # --- qtT = G^T @ xT : [D, S] bf16 ------------------------
                qtT = []
                for m in range(KD):
                    qtT.append(
                        wp.tile([P, S], BF, tag=f"qtT{m}", name=f"qtT{h}_{m}")
                    )
                for m in range(KD):
                    for s in range(NCH):
                        acc = pp.tile([P, CH], F32, tag="acc", bufs=3)
                        for kd in range(KD):
                            nc.tensor.matmul(
                                acc[:],
                                Gt[h][kd][:, m * P : (m + 1) * P],
                                xT[kd][:, s * CH : (s + 1) * CH],
                                start=(kd == 0),
                                stop=(kd == KD - 1),
                            )
                        nc.scalar.copy(qtT[m][:, s * CH : (s + 1) * CH], acc[:])

                # --- V' natural [S, D] + t3 bias column ------------------
                v = []
                for t_i in range(NT):
                    v.append(wp.tile([P, D], BF, tag=f"v{t_i}", name=f"v{h}_{t_i}"))
                t3 = wp.tile([P, NT], F32, tag="t3", name=f"t3_{h}", bufs=2)
                for t_i in range(NT):
                    acc = pp.tile([P, D], F32, tag="acc", bufs=3)
                    accb = pp.tile([P, 1], F32, tag="den", bufs=2)
                    for kd in range(KD):
                        nc.tensor.matmul(
                            acc[:],
                            xT[kd][:, t_i * P : (t_i + 1) * P],
                            Wvp[h][kd][:],
                            start=(kd == 0),
                            stop=(kd == KD - 1),
                        )
                        nc.tensor.matmul(
                            accb[:],
                            xT[kd][:, t_i * P : (t_i + 1) * P],
                            gcol[h][:, kd : kd + 1],
                            start=(kd == 0),
                            stop=(kd == KD - 1),
                        )
                    nc.vector.tensor_tensor(v[t_i][:], acc[:], bfvb[h][:], Alu.add)
                    nc.scalar.copy(t3[:, t_i : t_i + 1], accb[:])

                # --- attention over sq chunks ----------------------------
                for c in range(NCH):
                    etiles = []
                    for t_i in range(NT):
                        sacc = pp.tile([P, CH], F32, tag="acc", bufs=3)
                        for kd in range(KD):
                            nc.tensor.matmul(
                                sacc[:],
                                xT[kd][:, t_i * P : (t_i + 1) * P],
                                qtT[kd][:, c * CH : (c + 1) * CH],
                                start=(kd == 0),
                                stop=(kd == KD - 1),
                            )
                        et = wp.tile(
                            [P, CH], BF, tag="E", bufs=2 * NT, name=f"E{h}_{c}_{t_i}"
                        )
                        nc.scalar.activation(
                            et[:],
                            sacc[:],
                            Act.Exp,
                            bias=t3[:, t_i : t_i + 1],
                            scale=INV_SQRT_D,
                        )
                        etiles.append(et)

                    # PV natural + denominator, per 128-row sq tile
                    for j in range(CH // P):
                        st = c * (CH // P) + j
                        pv = pp.tile([P, D], F32, tag="pv", bufs=2)
                        den = pp.tile([P, 1], F32, tag="den", bufs=2)
                        for t_i in range(NT):
                            nc.tensor.matmul(
                                pv[:],
                                etiles[t_i][:, j * P : (j + 1) * P],
                                v[t_i][:],
                                start=(t_i == 0),
                                stop=(t_i == NT - 1),
                            )
                            nc.tensor.matmul(
                                den[:],
                                etiles[t_i][:, j * P : (j + 1) * P],
                                ones_sb[:],
                                start=(t_i == 0),
                                stop=(t_i == NT - 1),
                            )
                        invd = wp.tile([P, 1], F32, tag="invd", bufs=4)
                        nc.vector.reciprocal(invd[:], den[:])
                        if h == 0:
                            nc.vector.tensor_scalar_mul(out_acc[st][:], pv[:], invd[:])
                        else:
                            osb = wp.tile([P, D], F32, tag="osb", bufs=3)
                            nc.vector.scalar_tensor_tensor(
                                osb[:],
                                pv[:],
                                invd[:],
                                out_acc[st][:],
                                Alu.mult,
                                Alu.add,
                            )
                            nc.sync.dma_start(
                                out_d[st * P : (st + 1) * P, :], osb[:]
                            )

    nc.compile()
    return nc


def _get_nc():
    if "nc" not in _CACHE:
        _CACHE["nc"] = _build_nc()
    return _CACHE["nc"]


def _prep_inputs(x, W_sep, b_sep, W_multi, b_multi, W_res, b_res):
    """Host-side exact weight fusion (f64) + sharding + bf16 cast."""
    import ml_dtypes

    bf16 = ml_dtypes.bfloat16
    x = np.asarray(x, dtype=np.float32)
    W_sep = np.asarray(W_sep, dtype=np.float64)
    b_sep = np.asarray(b_sep, dtype=np.float64)
    W_multi = np.asarray(W_multi, dtype=np.float64)
    b_multi = np.asarray(b_multi, dtype=np.float64)
    W_res = np.asarray(W_res, dtype=np.float64)

    W_full = W_multi @ W_sep            # [3*D*H, D]
    b_full = W_multi @ b_sep + b_multi  # [3*D*H]
    Wq = W_full.reshape(H, E3, D)[:, 0:D, :]        # [H, D, D]
    Wk = W_full.reshape(H, E3, D)[:, D : 2 * D, :]
    Wv = W_full.reshape(H, E3, D)[:, 2 * D :, :]
    bq = b_full.reshape(H, E3)[:, 0:D]
    bv = b_full.reshape(H, E3)[:, 2 * D :]
    Wres_h = W_res.reshape(D, H, D).transpose(1, 0, 2)  # [H, dd, d]

    G = np.einsum("hdi,hdj->hij", Wq, Wk)               # [H, D(in), D(in)]
    WvpT = np.einsum("hvi,hdv->hid", Wv, Wres_h)        # [H, D(in), D(dd)]
    gvec = np.einsum("hdi,hd->hi", Wk, bq) * INV_SQRT_D  # [H, D(in)]
    bfv = np.einsum("hdv,hv->hd", Wres_h, bv)            # [H, D(dd)]

    xT = np.ascontiguousarray(x.transpose(0, 2, 1)).astype(bf16)  # [B, D, S]
    G = np.ascontiguousarray(G).astype(bf16)
    WvpT = np.ascontiguousarray(WvpT).astype(bf16)
    gvec = np.ascontiguousarray(gvec).astype(bf16)
    bfv = np.ascontiguousarray(bfv).astype(np.float32)

    in_maps = []
    for core in range(NC):
        b, hg = divmod(core, 4)
        sl = slice(2 * hg, 2 * hg + 2)
        in_maps.append(
            {
                "xT": xT[b],
                "G": np.ascontiguousarray(G[sl]),
                "WvpT": np.ascontiguousarray(WvpT[sl]),
                "gvec": np.ascontiguousarray(gvec[sl]),
                "bfv": np.ascontiguousarray(bfv[sl]),
            }
        )
    return in_maps


def kernel(x, W_sep, b_sep, W_multi, b_multi, W_res, b_res):
    from concourse.bass_utils import run_bass_kernel_spmd

    in_maps = _prep_inputs(x, W_sep, b_sep, W_multi, b_multi, W_res, b_res)
    nc = _get_nc()
    res = run_bass_kernel_spmd(nc, in_maps, list(range(NC)), trace=False)

    out = np.zeros((B, S, D), dtype=np.float32)
    for core in range(NC):
        out[core // 4] += res.results[core]["out"]
    out += np.asarray(b_res, dtype=np.float32)
    return out


# revision 13
# speedup vs baseline: 1.0301x; 1.0301x over previous
"""Trainium2 Bass kernel for nn_Attention (B=2, S=2048, D=512, H=8).

Sharding: 8 cores = 2 batches x 4 head-groups (2 heads each).

Algebraic fusions (exact, host-side weight preprocessing in f64):
  W_full = W_multi @ W_sep  (the two projection layers collapse into one),
  G_h    = Wq_h^T @ Wk_h    so  S_h = (x G_h) x^T   (k-projection vanishes),
  Wvp_h  = (Wres_h @ Wv_h)^T so PV directly yields the output partial
           (restore matmul vanishes).
Bias terms: q-side/constant bias cancels inside softmax; the surviving
k-side term (x Wk^T bq)/sqrt(D) is a per-sk vector folded into the exp's
per-partition bias; V'-bias is a broadcast add; b_res is added on host.

Device compute per core (bf16 matmuls, f32 PSUM, feature-on-partition
layouts, zero on-device transposes):
  qtT = G^T xT                      [D, S]   (per head)
  V'  = x Wvp (+bias)               [S, D]   natural layout
  t3  = x g                         [S]      (exp bias column)
  ST  = x qt^T  -> E = exp(ST/sqrt(D) + t3)  [sk, sq] chunks
  den = E^T ones ; out_h = (E^T V') * 1/den  accumulated over heads into
  out [S, D] f32 (natural layout), summed over cores on host + b_res.
"""

import numpy as np

P = 128
B = 2
S = 2048
D = 512          # word dim == head dim
H = 8            # total heads
E3 = 3 * D       # 1536
NHL = 2          # local heads per core
NC = 8           # cores
CH = 512         # sq chunk width
NCH = S // CH    # 4
NT = S // P      # 16 sk tiles
KD = D // P      # 4
INV_SQRT_D = 1.0 / float(np.sqrt(np.float32(D)))

_CACHE = {}


def _build_nc():
    import concourse.mybir as mybir
    import concourse.tile as tile
    from concourse import bacc

    dt = mybir.dt
    BF = dt.bfloat16
    F32 = dt.float32
    Act = mybir.ActivationFunctionType
    Alu = mybir.AluOpType

    nc = bacc.Bacc("TRN2", target_bir_lowering=False, debug=False, num_devices=NC)

    xT_d = nc.declare_dram_parameter("xT", [D, S], BF, isOutput=False)
    g_d = nc.declare_dram_parameter("G", [NHL, D, D], BF, isOutput=False)
    wvp_d = nc.declare_dram_parameter("WvpT", [NHL, D, D], BF, isOutput=False)
    gv_d = nc.declare_dram_parameter("gvec", [NHL, D], BF, isOutput=False)
    bfv_d = nc.declare_dram_parameter("bfv", [NHL, D], F32, isOutput=False)
    out_d = nc.declare_dram_parameter("out", [S, D], F32, isOutput=True)

    with tile.TileContext(nc) as tc:
        with (
            tc.tile_pool(name="w", bufs=1) as wp,
            tc.tile_pool(name="psum", bufs=1, space="PSUM") as pp,
        ):
            ones_sb = wp.tile([P, 1], BF, tag="ones")
            nc.vector.memset(ones_sb[:], 1.0)

            # xT lives as one [P, KD*S] tile (column block kd = d_in k-tile);
            # G / Wvp are one [P, KD*D] tile per head. Each DRAM load is then
            # a single multi-dim-AP DMA, and the first-needed ones issue
            # first (on different engines) so the PE can start early.
            xt = wp.tile([P, KD * S], BF, tag="xt", name="xt")
            gts, wvs, gcol = [], [], []
            for h in range(NHL):
                gts.append(wp.tile([P, KD * D], BF, tag=f"G{h}", name=f"G{h}"))
                wvs.append(wp.tile([P, KD * D], BF, tag=f"Wvp{h}", name=f"Wvp{h}"))
                gcol.append(wp.tile([P, KD], BF, tag=f"gcol{h}", name=f"gcol{h}"))

            def xts(kd, a, b):
                return xt[:, kd * S + a : kd * S + b]

            def gsl(h, kd, a, b):
                return gts[h][:, kd * D + a : kd * D + b]

            def wsl(h, kd):
                return wvs[h][:, kd * D : (kd + 1) * D]

            xT_dv = xT_d[:].rearrange("(k p) s -> p k s", p=P)
            xt_v = xt[:].rearrange("p (k s) -> p k s", k=KD)
            g_v = [g_d[h, :, :].rearrange("(k p) d -> p k d", p=P) for h in range(NHL)]
            gt_v = [gts[h][:].rearrange("p (k d) -> p k d", k=KD) for h in range(NHL)]
            wvp_v = [wvp_d[h, :, :].rearrange("(k p) d -> p k d", p=P) for h in range(NHL)]
            wv_v = [wvs[h][:].rearrange("p (k d) -> p k d", k=KD) for h in range(NHL)]

            nc.sync.dma_start(gt_v[0], g_v[0])
            for s in range(NCH):
                nc.gpsimd.dma_start(
                    xt_v[:, :, s * CH : (s + 1) * CH],
                    xT_dv[:, :, s * CH : (s + 1) * CH],
                )
            nc.scalar.dma_start(wv_v[0], wvp_v[0])
            nc.scalar.dma_start(
                gcol[0][:], gv_d[0, :].rearrange("(k p) -> p k", p=P)
            )
            nc.sync.dma_start(gt_v[1], g_v[1])
            nc.sync.dma_start(wv_v[1], wvp_v[1])
            nc.sync.dma_start(gcol[1][:], gv_d[1, :].rearrange("(k p) -> p k", p=P))
            bfvb = []
            for h in range(NHL):
                brow = wp.tile([1, D], F32, tag=f"bfvrow{h}", name=f"bfvrow{h}")
                nc.sync.dma_start(brow[:], bfv_d[h, :].unsqueeze(0))
                bb = wp.tile([P, D], F32, tag=f"bfvb{h}", name=f"bfvb{h}")
                nc.gpsimd.partition_broadcast(bb[:], brow[:])
                bfvb.append(bb)

            # out_acc: head-0 partial, f32 (16 tiles of [128, D])
            out_acc = []
            for st in range(NT):
                out_acc.append(
                    wp.tile([P, D], F32, tag=f"oacc{st}", name=f"oacc{st}")
                )

            for h in range(NHL):
                # --- qtT = G^T @ xT : [D, S] bf16 ------------------------
                qtT = []
                for m in range(KD):
                    qtT.append(
                        wp.tile([P, S], BF, tag=f"qtT{m}", name=f"qtT{h}_{m}")
                    )
                for s in range(NCH):
                    for m in range(KD):
                        acc = pp.tile([P, CH], F32, tag="acc", bufs=3)
                        for kd in range(KD):
                            nc.tensor.matmul(
                                acc[:],
                                gsl(h, kd, m * P, (m + 1) * P),
                                xts(kd, s * CH, (s + 1) * CH),
                                start=(kd == 0),
                                stop=(kd == KD - 1),
                            )
                        nc.scalar.copy(qtT[m][:, s * CH : (s + 1) * CH], acc[:])

                # --- V' natural [S, D] + t3 bias column ------------------
                v = []
                for t_i in range(NT):
                    v.append(wp.tile([P, D], BF, tag=f"v{t_i}", name=f"v{h}_{t_i}"))
                t3 = wp.tile([P, NT], F32, tag="t3", name=f"t3_{h}", bufs=2)
                for t_i in range(NT):
                    acc = pp.tile([P, D], F32, tag="acc", bufs=3)
                    accb = pp.tile([P, 1], F32, tag="den", bufs=2)
                    for kd in range(KD):
                        nc.tensor.matmul(
                            acc[:],
                            xts(kd, t_i * P, (t_i + 1) * P),
                            wsl(h, kd),
                            start=(kd == 0),
                            stop=(kd == KD - 1),
                        )
                        nc.tensor.matmul(
                            accb[:],
                            xts(kd, t_i * P, (t_i + 1) * P),
                            gcol[h][:, kd : kd + 1],
                            start=(kd == 0),
                            stop=(kd == KD - 1),
                        )
                    nc.vector.tensor_tensor(v[t_i][:], acc[:], bfvb[h][:], Alu.add)
                    nc.scalar.copy(t3[:, t_i : t_i + 1], accb[:])

                # --- attention over sq chunks ----------------------------
                for c in range(NCH):
                    etiles = []
                    for t_i in range(NT):
                        sacc = pp.tile([P, CH], F32, tag="acc", bufs=3)
                        for kd in range(KD):
                            nc.tensor.matmul(
                                sacc[:],
                                xts(kd, t_i * P, (t_i + 1) * P),
                                qtT[kd][:, c * CH : (c + 1) * CH],
                                start=(kd == 0),
                                stop=(kd == KD - 1),
                            )
                        et = wp.tile(
                            [P, CH], BF, tag="E", bufs=2 * NT, name=f"E{h}_{c}_{t_i}"
                        )
                        nc.scalar.activation(
                            et[:],
                            sacc[:],
                            Act.Exp,
                            bias=t3[:, t_i : t_i + 1],
                            scale=INV_SQRT_D,
                        )
                        etiles.append(et)

                    # PV natural + denominator, per 128-row sq tile
                    for j in range(CH // P):
                        st = c * (CH // P) + j
                        pv = pp.tile([P, D], F32, tag="pv", bufs=2)
                        den = pp.tile([P, 1], F32, tag="den", bufs=2)
                        for t_i in range(NT):
                            nc.tensor.matmul(
                                den[:],
                                etiles[t_i][:, j * P : (j + 1) * P],
                                ones_sb[:],
                                start=(t_i == 0),
                                stop=(t_i == NT - 1),
                            )
                            nc.tensor.matmul(
                                pv[:],
                                etiles[t_i][:, j * P : (j + 1) * P],
                                v[t_i][:],
                                start=(t_i == 0),
                                stop=(t_i == NT - 1),
                            )
                        invd = wp.tile([P, 1], F32, tag="invd", bufs=4)
                        nc.vector.reciprocal(invd[:], den[:])
                        if h == 0:
                            nc.vector.tensor_scalar_mul(out_acc[st][:], pv[:], invd[:])
                        else:
                            osb = wp.tile([P, D], F32, tag="osb", bufs=3)
                            nc.vector.scalar_tensor_tensor(
                                osb[:],
                                pv[:],
                                invd[:],
                                out_acc[st][:],
                                Alu.mult,
                                Alu.add,
                            )
                            nc.sync.dma_start(
                                out_d[st * P : (st + 1) * P, :], osb[:]
                            )

    nc.compile()
    return nc


def _get_nc():
    if "nc" not in _CACHE:
        _CACHE["nc"] = _build_nc()
    return _CACHE["nc"]


def _prep_inputs(x, W_sep, b_sep, W_multi, b_multi, W_res, b_res):
    """Host-side exact weight fusion (f64) + sharding + bf16 cast."""
    import ml_dtypes

    bf16 = ml_dtypes.bfloat16
    x = np.asarray(x, dtype=np.float32)
    W_sep = np.asarray(W_sep, dtype=np.float64)
    b_sep = np.asarray(b_sep, dtype=np.float64)
    W_multi = np.asarray(W_multi, dtype=np.float64)
    b_multi = np.asarray(b_multi, dtype=np.float64)
    W_res = np.asarray(W_res, dtype=np.float64)

    W_full = W_multi @ W_sep            # [3*D*H, D]
    b_full = W_multi @ b_sep + b_multi  # [3*D*H]
    Wq = W_full.reshape(H, E3, D)[:, 0:D, :]        # [H, D, D]
    Wk = W_full.reshape(H, E3, D)[:, D : 2 * D, :]
    Wv = W_full.reshape(H, E3, D)[:, 2 * D :, :]
    bq = b_full.reshape(H, E3)[:, 0:D]
    bv = b_full.reshape(H, E3)[:, 2 * D :]
    Wres_h = W_res.reshape(D, H, D).transpose(1, 0, 2)  # [H, dd, d]

    G = np.einsum("hdi,hdj->hij", Wq, Wk)               # [H, D(in), D(in)]
    WvpT = np.einsum("hvi,hdv->hid", Wv, Wres_h)        # [H, D(in), D(dd)]
    gvec = np.einsum("hdi,hd->hi", Wk, bq) * INV_SQRT_D  # [H, D(in)]
    bfv = np.einsum("hdv,hv->hd", Wres_h, bv)            # [H, D(dd)]

    xT = np.ascontiguousarray(x.transpose(0, 2, 1)).astype(bf16)  # [B, D, S]
    G = np.ascontiguousarray(G).astype(bf16)
    WvpT = np.ascontiguousarray(WvpT).astype(bf16)
    gvec = np.ascontiguousarray(gvec).astype(bf16)
    bfv = np.ascontiguousarray(bfv).astype(np.float32)

    in_maps = []
    for core in range(NC):
        b, hg = divmod(core, 4)
        sl = slice(2 * hg, 2 * hg + 2)
        in_maps.append(
            {
                "xT": xT[b],
                "G": np.ascontiguousarray(G[sl]),
                "WvpT": np.ascontiguousarray(WvpT[sl]),
                "gvec": np.ascontiguousarray(gvec[sl]),
                "bfv": np.ascontiguousarray(bfv[sl]),
            }
        )
    return in_maps


def kernel(x, W_sep, b_sep, W_multi, b_multi, W_res, b_res):
    from concourse.bass_utils import run_bass_kernel_spmd

    in_maps = _prep_inputs(x, W_sep, b_sep, W_multi, b_multi, W_res, b_res)
    nc = _get_nc()
    res = run_bass_kernel_spmd(nc, in_maps, list(range(NC)), trace=False)

    out = np.zeros((B, S, D), dtype=np.float32)
    for core in range(NC):
        out[core // 4] += res.results[core]["out"]
    out += np.asarray(b_res, dtype=np.float32)
    return out


# revision 19
# speedup vs baseline: 1.0355x; 1.0053x over previous
"""Trainium2 Bass kernel for nn_Attention (B=2, S=2048, D=512, H=8).

Sharding: 8 cores = 2 batches x 4 head-groups (2 heads each).

Algebraic fusions (exact, host-side weight preprocessing in f64):
  W_full = W_multi @ W_sep  (the two projection layers collapse into one),
  G_h    = Wq_h^T @ Wk_h    so  S_h = (x G_h) x^T   (k-projection vanishes),
  Wvp_h  = (Wres_h @ Wv_h)^T so PV directly yields the output partial
           (restore matmul vanishes).
Bias terms: q-side/constant bias cancels inside softmax; the surviving
k-side term (x Wk^T bq)/sqrt(D) is a per-sk vector folded into the exp's
per-partition bias; V'-bias is a broadcast add; b_res is added on host.

Device compute per core (bf16 matmuls, f32 PSUM, feature-on-partition
layouts, zero on-device transposes):
  qtT = G^T xT                      [D, S]   (per head)
  V'  = x Wvp (+bias)               [S, D]   natural layout
  t3  = x g                         [S]      (exp bias column)
  ST  = x qt^T  -> E = exp(ST/sqrt(D) + t3)  [sk, sq] chunks
  den = E^T ones ; out_h = (E^T V') * 1/den  accumulated over heads into
  out [S, D] f32 (natural layout), summed over cores on host + b_res.
"""

import numpy as np

P = 128
B = 2
S = 2048
D = 512          # word dim == head dim
H = 8            # total heads
E3 = 3 * D       # 1536
NHL = 2          # local heads per core
NC = 8           # cores
CH = 512         # sq chunk width
NCH = S // CH    # 4
NT = S // P      # 16 sk tiles
KD = D // P      # 4
INV_SQRT_D = 1.0 / float(np.sqrt(np.float32(D)))

_CACHE = {}


def _build_nc():
    import concourse.mybir as mybir
    import concourse.tile as tile
    from concourse import bacc

    dt = mybir.dt
    BF = dt.bfloat16
    F32 = dt.float32
    Act = mybir.ActivationFunctionType
    Alu = mybir.AluOpType

    nc = bacc.Bacc("TRN2", target_bir_lowering=False, debug=False, num_devices=NC)

    xT_d = nc.declare_dram_parameter("xT", [D, S], BF, isOutput=False)
    g_d = nc.declare_dram_parameter("G", [NHL, D, D], BF, isOutput=False)
    wvp_d = nc.declare_dram_parameter("WvpT", [NHL, D, D], BF, isOutput=False)
    gv_d = nc.declare_dram_parameter("gvec", [NHL, D], BF, isOutput=False)
    bfv_d = nc.declare_dram_parameter("bfv", [NHL, D], F32, isOutput=False)
    out_d = nc.declare_dram_parameter("out", [S, D], F32, isOutput=True)

    with tile.TileContext(nc) as tc:
        with (
            tc.tile_pool(name="w", bufs=1) as wp,
            tc.tile_pool(name="psum", bufs=1, space="PSUM") as pp,
        ):
            ones_sb = wp.tile([P, 1], BF, tag="ones")
            nc.vector.memset(ones_sb[:], 1.0)

            # xT lives as one [P, KD*S] tile (column block kd = d_in k-tile);
            # G / Wvp are one [P, KD*D] tile per head. Each DRAM load is then
            # a single multi-dim-AP DMA, and the first-needed ones issue
            # first (on different engines) so the PE can start early.
            xt = wp.tile([P, KD * S], BF, tag="xt", name="xt")
            gts, wvs, gcol = [], [], []
            for h in range(NHL):
                gts.append(wp.tile([P, KD * D], BF, tag=f"G{h}", name=f"G{h}"))
                wvs.append(wp.tile([P, KD * D], BF, tag=f"Wvp{h}", name=f"Wvp{h}"))
                gcol.append(wp.tile([P, KD], BF, tag=f"gcol{h}", name=f"gcol{h}"))

            def xts(kd, a, b):
                return xt[:, kd * S + a : kd * S + b]

            def gsl(h, kd, a, b):
                return gts[h][:, kd * D + a : kd * D + b]

            def wsl(h, kd):
                return wvs[h][:, kd * D : (kd + 1) * D]

            xT_dv = xT_d[:].rearrange("(k p) s -> p k s", p=P)
            xt_v = xt[:].rearrange("p (k s) -> p k s", k=KD)
            g_v = [g_d[h, :, :].rearrange("(k p) d -> p k d", p=P) for h in range(NHL)]
            gt_v = [gts[h][:].rearrange("p (k d) -> p k d", k=KD) for h in range(NHL)]
            wvp_v = [wvp_d[h, :, :].rearrange("(k p) d -> p k d", p=P) for h in range(NHL)]
            wv_v = [wvs[h][:].rearrange("p (k d) -> p k d", k=KD) for h in range(NHL)]

            # first-needed strips first: G0 columns for m=0, then a half
            # xT chunk, so the first q~ psum group's inputs land ~1us sooner
            nc.sync.dma_start(gt_v[0][:, :, 0:P], g_v[0][:, :, 0:P])
            nc.gpsimd.dma_start(xt_v[:, :, 0 : CH // 2], xT_dv[:, :, 0 : CH // 2])
            nc.sync.dma_start(gt_v[0][:, :, P:D], g_v[0][:, :, P:D])
            nc.gpsimd.dma_start(
                xt_v[:, :, CH // 2 : CH], xT_dv[:, :, CH // 2 : CH]
            )
            for s in range(1, NCH):
                nc.gpsimd.dma_start(
                    xt_v[:, :, s * CH : (s + 1) * CH],
                    xT_dv[:, :, s * CH : (s + 1) * CH],
                )
            nc.sync.dma_start(wv_v[0], wvp_v[0])
            nc.sync.dma_start(
                gcol[0][:], gv_d[0, :].rearrange("(k p) -> p k", p=P)
            )
            nc.sync.dma_start(gt_v[1], g_v[1])
            nc.sync.dma_start(wv_v[1], wvp_v[1])
            nc.sync.dma_start(gcol[1][:], gv_d[1, :].rearrange("(k p) -> p k", p=P))
            bfvb = []
            for h in range(NHL):
                brow = wp.tile([1, D], F32, tag=f"bfvrow{h}", name=f"bfvrow{h}")
                nc.sync.dma_start(brow[:], bfv_d[h, :].unsqueeze(0))
                bb = wp.tile([P, D], F32, tag=f"bfvb{h}", name=f"bfvb{h}")
                nc.gpsimd.partition_broadcast(bb[:], brow[:])
                bfvb.append(bb)

            # out_acc: head-0 partial, f32 (16 tiles of [128, D])
            out_acc = []
            for st in range(NT):
                out_acc.append(
                    wp.tile([P, D], F32, tag=f"oacc{st}", name=f"oacc{st}")
                )

            for h in range(NHL):
                # --- qtT = G^T @ xT : [D, S] bf16 ------------------------
                qtT = []
                for m in range(KD):
                    qtT.append(
                        wp.tile([P, S], BF, tag=f"qtT{m}", name=f"qtT{h}_{m}")
                    )
                for s in range(NCH):
                    for m in range(KD):
                        acc = pp.tile([P, CH], F32, tag="acc", bufs=3)
                        if h == 0 and s == 0 and m == 0:
                            for half in range(2):
                                a0, a1 = half * CH // 2, (half + 1) * CH // 2
                                for kd in range(KD):
                                    nc.tensor.matmul(
                                        acc[:, a0:a1],
                                        gsl(h, kd, 0, P),
                                        xts(kd, a0, a1),
                                        start=(kd == 0),
                                        stop=(kd == KD - 1),
                                    )
                        else:
                            for kd in range(KD):
                                nc.tensor.matmul(
                                    acc[:],
                                    gsl(h, kd, m * P, (m + 1) * P),
                                    xts(kd, s * CH, (s + 1) * CH),
                                    start=(kd == 0),
                                    stop=(kd == KD - 1),
                                )
                        nc.scalar.copy(qtT[m][:, s * CH : (s + 1) * CH], acc[:])

                # --- V' natural [S, D] + t3 bias column ------------------
                v = []
                for t_i in range(NT):
                    v.append(wp.tile([P, D], BF, tag=f"v{t_i}", name=f"v{h}_{t_i}"))
                t3 = wp.tile([P, NT], F32, tag="t3", name=f"t3_{h}", bufs=2)
                for t_i in range(NT):
                    acc = pp.tile([P, D], F32, tag="acc", bufs=3)
                    accb = pp.tile([P, 1], F32, tag="den", bufs=2)
                    for kd in range(KD):
                        nc.tensor.matmul(
                            acc[:],
                            xts(kd, t_i * P, (t_i + 1) * P),
                            wsl(h, kd),
                            start=(kd == 0),
                            stop=(kd == KD - 1),
                        )
                        nc.tensor.matmul(
                            accb[:],
                            xts(kd, t_i * P, (t_i + 1) * P),
                            gcol[h][:, kd : kd + 1],
                            start=(kd == 0),
                            stop=(kd == KD - 1),
                        )
                    nc.vector.tensor_tensor(v[t_i][:], acc[:], bfvb[h][:], Alu.add)
                    nc.scalar.copy(t3[:, t_i : t_i + 1], accb[:])

                # --- attention over sq chunks ----------------------------
                for c in range(NCH):
                    etiles = []
                    for t_i in range(NT):
                        sacc = pp.tile([P, CH], F32, tag="acc", bufs=3)
                        for kd in range(KD):
                            nc.tensor.matmul(
                                sacc[:],
                                xts(kd, t_i * P, (t_i + 1) * P),
                                qtT[kd][:, c * CH : (c + 1) * CH],
                                start=(kd == 0),
                                stop=(kd == KD - 1),
                            )
                        et = wp.tile(
                            [P, CH], BF, tag="E", bufs=2 * NT, name=f"E{h}_{c}_{t_i}"
                        )
                        nc.scalar.activation(
                            et[:],
                            sacc[:],
                            Act.Exp,
                            bias=t3[:, t_i : t_i + 1],
                            scale=INV_SQRT_D,
                        )
                        etiles.append(et)

                    # PV natural + denominator, per 128-row sq tile
                    for j in range(CH // P):
                        st = c * (CH // P) + j
                        pv = pp.tile([P, D], F32, tag="pv", bufs=2)
                        den = pp.tile([P, 1], F32, tag="den", bufs=2)
                        for t_i in range(NT):
                            nc.tensor.matmul(
                                den[:],
                                etiles[t_i][:, j * P : (j + 1) * P],
                                ones_sb[:],
                                start=(t_i == 0),
                                stop=(t_i == NT - 1),
                            )
                            nc.tensor.matmul(
                                pv[:],
                                etiles[t_i][:, j * P : (j + 1) * P],
                                v[t_i][:],
                                start=(t_i == 0),
                                stop=(t_i == NT - 1),
                            )
                        invd = wp.tile([P, 1], F32, tag="invd", bufs=4)
                        nc.vector.reciprocal(invd[:], den[:])
                        if h == 0:
                            nc.vector.tensor_scalar_mul(out_acc[st][:], pv[:], invd[:])
                        else:
                            osb = wp.tile([P, D], F32, tag="osb", bufs=3)
                            nc.vector.scalar_tensor_tensor(
                                osb[:],
                                pv[:],
                                invd[:],
                                out_acc[st][:],
                                Alu.mult,
                                Alu.add,
                            )
                            nc.sync.dma_start(
                                out_d[st * P : (st + 1) * P, :], osb[:]
                            )

    nc.compile()
    return nc


def _get_nc():
    if "nc" not in _CACHE:
        _CACHE["nc"] = _build_nc()
    return _CACHE["nc"]


def _prep_inputs(x, W_sep, b_sep, W_multi, b_multi, W_res, b_res):
    """Host-side exact weight fusion (f64) + sharding + bf16 cast."""
    import ml_dtypes

    bf16 = ml_dtypes.bfloat16
    x = np.asarray(x, dtype=np.float32)
    W_sep = np.asarray(W_sep, dtype=np.float64)
    b_sep = np.asarray(b_sep, dtype=np.float64)
    W_multi = np.asarray(W_multi, dtype=np.float64)
    b_multi = np.asarray(b_multi, dtype=np.float64)
    W_res = np.asarray(W_res, dtype=np.float64)

    W_full = W_multi @ W_sep            # [3*D*H, D]
    b_full = W_multi @ b_sep + b_multi  # [3*D*H]
    Wq = W_full.reshape(H, E3, D)[:, 0:D, :]        # [H, D, D]
    Wk = W_full.reshape(H, E3, D)[:, D : 2 * D, :]
    Wv = W_full.reshape(H, E3, D)[:, 2 * D :, :]
    bq = b_full.reshape(H, E3)[:, 0:D]
    bv = b_full.reshape(H, E3)[:, 2 * D :]
    Wres_h = W_res.reshape(D, H, D).transpose(1, 0, 2)  # [H, dd, d]

    G = np.einsum("hdi,hdj->hij", Wq, Wk)               # [H, D(in), D(in)]
    WvpT = np.einsum("hvi,hdv->hid", Wv, Wres_h)        # [H, D(in), D(dd)]
    gvec = np.einsum("hdi,hd->hi", Wk, bq) * INV_SQRT_D  # [H, D(in)]
    bfv = np.einsum("hdv,hv->hd", Wres_h, bv)            # [H, D(dd)]

    xT = np.ascontiguousarray(x.transpose(0, 2, 1)).astype(bf16)  # [B, D, S]
    G = np.ascontiguousarray(G).astype(bf16)
    WvpT = np.ascontiguousarray(WvpT).astype(bf16)
    gvec = np.ascontiguousarray(gvec).astype(bf16)
    bfv = np.ascontiguousarray(bfv).astype(np.float32)

    in_maps = []
    for core in range(NC):
        b, hg = divmod(core, 4)
        sl = slice(2 * hg, 2 * hg + 2)
        in_maps.append(
            {
                "xT": xT[b],
                "G": np.ascontiguousarray(G[sl]),
                "WvpT": np.ascontiguousarray(WvpT[sl]),
                "gvec": np.ascontiguousarray(gvec[sl]),
                "bfv": np.ascontiguousarray(bfv[sl]),
            }
        )
    return in_maps


def kernel(x, W_sep, b_sep, W_multi, b_multi, W_res, b_res):
    from concourse.bass_utils import run_bass_kernel_spmd

    in_maps = _prep_inputs(x, W_sep, b_sep, W_multi, b_multi, W_res, b_res)
    nc = _get_nc()
    res = run_bass_kernel_spmd(nc, in_maps, list(range(NC)), trace=False)

    out = np.zeros((B, S, D), dtype=np.float32)
    for core in range(NC):
        out[core // 4] += res.results[core]["out"]
    out += np.asarray(b_res, dtype=np.float32)
    return out
